# revision 13
# baseline (speedup 1.0000x reference)
"""TRN2 Bass kernel for nn_Attention_59081570125142.

MobileViT-style attention block:
  qkv = BN(1x1conv(x)); per-head attention over N=1024 tokens
  (key_dim=16, head_dim=32, 8 heads); pos_enc = BN(dwconv3x3(v));
  out = BN(1x1conv(v_attn + pos_enc)).

Sharding: data-parallel over batch B=16 across 8 cores (2 images/core).
All BN folded into conv weights/biases on host. Matmuls in bf16
(measured end-to-end rel err ~3e-3), accumulation fp32 in PSUM.

Softmax layout trick: logits computed KEY-major (logitsT[k, q] via
lhsT=k_head, rhs=q_head) so softmax normalizer is a column sum that
rides for free as a ones-column appended to v^T in the AV matmul
(out rows 0..31 = unnormalized AV, row 32 = denominator).
QK packs 4 heads in the PE array via row tiling (K=16 each);
AV packs 2 heads via col tiling (M=33 at psum partition 0/64).
"""
import sys

sys.path.insert(0, '/opt/trn_rl_repo')

import numpy as np
import ml_dtypes

import concourse.bass as bass
import concourse.mybir as mybir
from concourse import tile
from concourse.bass_utils import run_bass_kernel_spmd

F32 = mybir.dt.float32
F32R = mybir.dt.float32r
BF16 = mybir.dt.bfloat16
AF = mybir.ActivationFunctionType
ALU = mybir.AluOpType

N_CORES = 8
B = 16
B_LOC = B // N_CORES          # 2 images per core
DIM = 256
NUM_HEADS = 8
HEAD_DIM = 32
KEY_DIM = 16
QKV_OUT = 512
N = 1024                      # 32*32 tokens
H = W = 32
SCALE = KEY_DIM ** -0.5       # 0.25
BN_EPS = 1e-3

_cache = {}


def _fold_bn(w, gamma, beta, mean, var):
    inv = gamma / np.sqrt(var + BN_EPS)
    return w * inv[:, None], beta - mean * inv


def _prep_weights(qkv_w, qkv_gamma, qkv_beta, qkv_mean, qkv_var,
                  pe_w, pe_gamma, pe_beta, pe_mean, pe_var,
                  proj_w, proj_gamma, proj_beta, proj_mean, proj_var):
    """Host-side BN folding + layout rearrangement."""
    W1, b1 = _fold_bn(qkv_w[:, :, 0, 0].astype(np.float32), qkv_gamma, qkv_beta,
                      qkv_mean, qkv_var)                      # (512, 256), (512,)
    W2, b2 = _fold_bn(proj_w[:, :, 0, 0].astype(np.float32), proj_gamma, proj_beta,
                      proj_mean, proj_var)                    # (256, 256), (256,)
    invpe = pe_gamma / np.sqrt(pe_var + BN_EPS)
    PW = (pe_w[:, 0] * invpe[:, None, None]).astype(np.float32)  # (256, 3, 3)
    bpe = (pe_beta - pe_mean * invpe).astype(np.float32)         # (256,)

    # channel-major qkv pass: out = W1T_cm.T @ x, 6 output tiles of 128:
    #  t0/t1: K heads 0-3 / 4-7 at 32-aligned slots (16 rows used per head)
    #  t2/t3: Q likewise;  t4/t5: V natural order (head h -> rows 32h..32h+32)
    w1t_cm = np.zeros((256, 768), np.float32)
    b1_cm = np.zeros((768,), np.float32)
    for h in range(NUM_HEADS):
        t = h // 4
        j = h % 4
        # K
        cols = t * 128 + 32 * j + np.arange(16)
        chans = h * 64 + 16 + np.arange(16)
        w1t_cm[:, cols] = W1[chans].T
        b1_cm[cols] = b1[chans]
        # Q
        cols = 256 + t * 128 + 32 * j + np.arange(16)
        chans = h * 64 + np.arange(16)
        w1t_cm[:, cols] = W1[chans].T
        b1_cm[cols] = b1[chans]
        # V
        cols = 512 + h * 32 + np.arange(32)
        chans = h * 64 + 32 + np.arange(32)
        w1t_cm[:, cols] = W1[chans].T
        b1_cm[cols] = b1[chans]

    # token-major v pass: vT[tok, 33h+d] = sum_c x[c,tok] * W1'[h*64+32+d, c];
    # col 33h+32 gets 0 from the matmul and 1.0 from the bias tile (ones col
    # for the fused softmax denominator).
    wvt = np.zeros((256, 264), np.float32)
    bvb = np.zeros((264,), np.float32)
    for h in range(NUM_HEADS):
        cols = 33 * h + np.arange(32)
        chans = h * 64 + 32 + np.arange(32)
        wvt[:, cols] = W1[chans].T
        bvb[cols] = b1[chans]
        bvb[33 * h + 32] = 1.0

    bf = ml_dtypes.bfloat16
    return dict(
        w1t=w1t_cm.astype(bf),                  # (256, 768)
        b1=b1_cm.reshape(768, 1).astype(np.float32),
        wvt=wvt.astype(bf),                     # (256, 264)
        bvb=np.broadcast_to(bvb, (128, 264)).copy().astype(np.float32),
        w2t=W2.T.copy().astype(bf),             # (256, 256)
        b2=b2.reshape(256, 1).astype(np.float32),
        pew=PW.reshape(256, 9).astype(np.float32),
        bpe=bpe.reshape(256, 1).astype(np.float32),
    )


def _split_waits(nc, cap=1):
    """Walrus rejects instructions with more than ~1-2 semaphore waits
    ("Too many sync wait commands"). Hoist excess waits onto same-engine
    EventSemaphore carrier instructions inserted just before the offender —
    the engine stalls at the carriers first, semantically identical."""
    n = 0
    for f in nc.m.functions:
        for blk in f.blocks:
            insts = blk.instructions
            out = []
            for inst in insts:
                si = inst.sync_info
                waits = list(si.on_wait) if si else []
                if len(waits) > cap and inst.opcode != "EventSemaphore":
                    extra, keep = waits[:-cap], waits[-cap:]
                    for k, w in enumerate(extra):
                        ev = mybir.InstEventSemaphore(
                            name=f"{inst.name}-sw{k}", ins=[], outs=[],
                            sync_info=mybir.SyncInfo(on_wait=[w], on_update=[]),
                        )
                        ev.engine = inst.engine
                        out.append(ev)
                        n += 1
                    inst.sync_info = mybir.SyncInfo(
                        on_wait=keep, on_update=list(si.on_update))
                out.append(inst)
            blk.instructions = out
    return n


def _build_program():
    nc = bass.Bass()
    x_d = nc.declare_dram_parameter("x", [B_LOC, 256, N], BF16, isOutput=False)
    w1t_d = nc.declare_dram_parameter("w1t", [256, 768], BF16, isOutput=False)
    b1_d = nc.declare_dram_parameter("b1", [768, 1], F32, isOutput=False)
    wvt_d = nc.declare_dram_parameter("wvt", [256, 264], BF16, isOutput=False)
    bvb_d = nc.declare_dram_parameter("bvb", [128, 264], F32, isOutput=False)
    w2t_d = nc.declare_dram_parameter("w2t", [256, 256], BF16, isOutput=False)
    b2_d = nc.declare_dram_parameter("b2", [256, 1], F32, isOutput=False)
    pew_d = nc.declare_dram_parameter("pew", [256, 9], F32, isOutput=False)
    bpe_d = nc.declare_dram_parameter("bpe", [256, 1], F32, isOutput=False)
    out_d = nc.declare_dram_parameter("out", [B_LOC, 256, N], F32, isOutput=True)

    with tile.TileContext(nc) as tc:
        with (
            tc.tile_pool(name="const", bufs=1) as cpool,
            tc.tile_pool(name="xp", bufs=1) as xpool,
            tc.tile_pool(name="qkv", bufs=1) as qkvpool,
            tc.tile_pool(name="vt", bufs=1) as vtpool,
            tc.tile_pool(name="pe", bufs=1) as pepool,
            tc.tile_pool(name="petmp", bufs=2) as petmp,
            tc.tile_pool(name="exp", bufs=3) as exppool,
            tc.tile_pool(name="attn", bufs=1) as attnpool,
            tc.tile_pool(name="nrm", bufs=2) as nrmpool,
            tc.tile_pool(name="outp", bufs=2) as outpool,
            tc.tile_pool(name="ps_big", bufs=1, space="PSUM") as ps_big,     # 1 bank
            tc.tile_pool(name="ps_small", bufs=1, space="PSUM") as ps_small, # 1 bank (vt + bcast)
            tc.tile_pool(name="ps_qk", bufs=2, space="PSUM") as ps_qk,       # 4 banks
            tc.tile_pool(name="ps_av", bufs=2, space="PSUM") as ps_av,       # 2 banks
        ):
            # ---- load constants ----
            w1t = [cpool.tile([128, 768], BF16, tag=f"w1t{i}", name=f"w1t{i}") for i in range(2)]
            wvt = [cpool.tile([128, 264], BF16, tag=f"wvt{i}", name=f"wvt{i}") for i in range(2)]
            w2t = [cpool.tile([128, 256], BF16, tag=f"w2t{i}", name=f"w2t{i}") for i in range(2)]
            b2 = [cpool.tile([128, 1], F32, tag=f"b2{i}", name=f"b2{i}") for i in range(2)]
            pew = [cpool.tile([128, 9], F32, tag=f"pew{i}", name=f"pew{i}") for i in range(2)]
            bpe = [cpool.tile([128, 1], F32, tag=f"bpe{i}", name=f"bpe{i}") for i in range(2)]
            b1 = [cpool.tile([128, 1], F32, tag=f"b1{i}", name=f"b1{i}") for i in range(6)]
            bvb = cpool.tile([128, 264], F32, tag="bvb")
            for ct in range(2):
                nc.sync.dma_start(w1t[ct][:], w1t_d[128 * ct:128 * (ct + 1), :])
                nc.sync.dma_start(wvt[ct][:], wvt_d[128 * ct:128 * (ct + 1), :])
                nc.sync.dma_start(w2t[ct][:], w2t_d[128 * ct:128 * (ct + 1), :])
                nc.sync.dma_start(b2[ct][:], b2_d[128 * ct:128 * (ct + 1), :])
                nc.sync.dma_start(pew[ct][:], pew_d[128 * ct:128 * (ct + 1), :])
                nc.sync.dma_start(bpe[ct][:], bpe_d[128 * ct:128 * (ct + 1), :])
            for ot in range(6):
                nc.sync.dma_start(b1[ot][:], b1_d[128 * ot:128 * (ot + 1), :])
            nc.sync.dma_start(bvb[:], bvb_d[:])
            ones32 = cpool.tile([1, 32], BF16, tag="ones32", name="ones32")
            nc.vector.memset(ones32[:], 1.0)

            xs = [[xpool.tile([128, N], BF16, tag=f"x{im}{ct}", name=f"x{im}{ct}") for ct in range(2)]
                  for im in range(B_LOC)]
            for im in range(B_LOC):
                for ct in range(2):
                    nc.sync.dma_start(xs[im][ct][:], x_d[im, 128 * ct:128 * (ct + 1), :])

            # ================= phase 1: preprocessing (both images) ======
            qkv_all, vts_all, peacc_all = [], [], []
            for im in range(B_LOC):
                # ---- qkv pass (channel-major K/Q/V tiles) ----
                qkv = [qkvpool.tile([128, N], BF16, tag=f"qkv{im}{ot}",
                                    name=f"qkv{im}{ot}") for ot in range(6)]
                for ot in range(6):
                    for ncol in range(2):
                        pq = ps_big.tile([128, 512], F32, tag="big", name="pq")
                        for ct in range(2):
                            nc.tensor.matmul(
                                pq[:],
                                w1t[ct][:, 128 * ot:128 * (ot + 1)],
                                xs[im][ct][:, 512 * ncol:512 * (ncol + 1)],
                                start=(ct == 0), stop=(ct == 1),
                            )
                        nc.vector.tensor_scalar_add(
                            qkv[ot][:, 512 * ncol:512 * (ncol + 1)], pq[:],
                            b1[ot][:])
                qkv_all.append(qkv)

                # ---- token-major v^T (+ones col) pass ----
                vts = [vtpool.tile([128, 264], BF16, tag=f"vt{im}{tt}",
                                   name=f"vt{im}{tt}") for tt in range(8)]
                for tt in range(8):
                    pv = ps_small.tile([128, 512], F32, tag="small", name="pv")
                    for ct in range(2):
                        nc.tensor.matmul(
                            pv[:, 0:264],
                            xs[im][ct][:, 128 * tt:128 * (tt + 1)], wvt[ct][:],
                            start=(ct == 0), stop=(ct == 1),
                        )
                    nc.vector.tensor_tensor(vts[tt][:], pv[:, 0:264], bvb[:],
                                            op=ALU.add)
                vts_all.append(vts)

                # ---- depthwise 3x3 conv on v (positional encoding) ----
                peacc = []
                for ct in range(2):
                    vpad = petmp.tile([128, 34, 34], BF16, tag="vpad",
                                      name="vpad")
                    nc.vector.memset(vpad[:], 0.0)
                    vimg = qkv[4 + ct][:].rearrange("p (h w) -> p h w", h=H)
                    nc.vector.tensor_copy(vpad[:, 1:33, 1:33], vimg)
                    acc_a = pepool.tile([128, N], BF16, tag=f"peacc{im}{ct}",
                                        name=f"peacc{im}{ct}")
                    acc_b = petmp.tile([128, N], BF16, tag="peacc_tmp",
                                       name="peacc_tmp")
                    a2d = acc_a[:].rearrange("p (h w) -> p h w", h=H)
                    b2d = acc_b[:].rearrange("p (h w) -> p h w", h=H)
                    nc.vector.tensor_scalar(
                        a2d, vpad[:, 1:33, 1:33], pew[ct][:, 4:5], bpe[ct][:],
                        op0=ALU.mult, op1=ALU.add,
                    )
                    taps = [(ky, kx) for ky in range(3) for kx in range(3)
                            if not (ky == 1 and kx == 1)]
                    cur, nxt = a2d, b2d
                    for ky, kx in taps:
                        nc.vector.scalar_tensor_tensor(
                            nxt, vpad[:, ky:ky + 32, kx:kx + 32],
                            pew[ct][:, (3 * ky + kx):(3 * ky + kx) + 1], cur,
                            op0=ALU.mult, op1=ALU.add,
                        )
                        cur, nxt = nxt, cur
                    peacc.append(cur)
                peacc_all.append(peacc)

            # ================= phase 2: attention, software-pipelined =====
            # 2-head groups g: heads A=2g, B=2g+1 share K/Q tile t=g//2.
            # QK row-tiled (K=16 at 32-aligned array rows), one PSUM BANK per
            # head (concurrent row-tiled matmuls must hit different banks).
            # AV col-tiled at psum partitions 0/64 of one bank.
            # The kt loop is pipelined: QK(kt+1) is issued BEFORE AV(kt) so
            # the in-order PE never makes ACT wait, and each group's
            # normalization is deferred into the next group's kt loop.
            attn_all = [[attnpool.tile([128, N], BF16, tag=f"attn{im}{ct}",
                                       name=f"attn{im}{ct}")
                         for ct in range(2)] for im in range(B_LOC)]
            seq = [(im, g, qc) for im in range(B_LOC)
                   for g in range(4) for qc in range(2)]
            pqk_store = {}

            def emit_qk(i, kt):
                im, g, qc = seq[i]
                t = g // 2
                jA = (2 * g) % 4
                pqk = ps_qk.tile([128, 1024], F32, tag="qk", name="pqk")
                for j, bank in ((jA, 0), (jA + 1, 1)):
                    nc.tensor.matmul(
                        pqk[:, 512 * bank:512 * (bank + 1)],
                        qkv_all[im][t][32 * j:32 * j + 16,
                                       128 * kt:128 * (kt + 1)],
                        qkv_all[im][2 + t][32 * j:32 * j + 16,
                                           512 * qc:512 * (qc + 1)],
                        tile_position=(32 * j, 0),
                    )
                pqk_store[(i, kt)] = pqk

            def emit_norm(i, pav):
                im, g, qc = seq[i]
                rec = nrmpool.tile([1, 1024], BF16, tag="rec", name="rec")
                with nc.allow_low_precision(reason="bf16 softmax den"):
                    nc.vector.reciprocal(rec[:, 0:512], pav[32:33, :])
                    nc.vector.reciprocal(rec[:, 512:1024], pav[96:97, :])
                pbc = ps_small.tile([128, 512], F32, tag="small", name="pbc")
                nc.tensor.matmul(pbc[0:32, :], ones32[:], rec[:, 0:512],
                                 tile_position=(0, 0))
                nc.tensor.matmul(pbc[64:96, :], ones32[:], rec[:, 512:1024],
                                 tile_position=(0, 64))
                rb = nrmpool.tile([128, 512], F32, tag="rb", name="rb")
                nc.vector.tensor_copy(rb[:], pbc[:])
                ct = g // 2
                rbase = (g % 2) * 64
                nc.vector.tensor_tensor(
                    attn_all[im][ct][rbase:rbase + 32,
                                     512 * qc:512 * (qc + 1)],
                    pav[0:32, :], rb[0:32, :], op=ALU.mult)
                nc.vector.tensor_tensor(
                    attn_all[im][ct][rbase + 32:rbase + 64,
                                     512 * qc:512 * (qc + 1)],
                    pav[64:96, :], rb[64:96, :], op=ALU.mult)

            emit_qk(0, 0)
            norm_pending = []
            for i in range(len(seq)):
                im, g, qc = seq[i]
                hA, hB = 2 * g, 2 * g + 1
                pav = ps_av.tile([128, 512], F32, tag="av", name="pav")
                for kt in range(8):
                    est = exppool.tile([128, 1024], BF16, tag="est",
                                       name="est")
                    nc.scalar.activation(est[:], pqk_store.pop((i, kt))[:],
                                         AF.Exp, scale=SCALE)
                    if kt + 1 < 8:
                        emit_qk(i, kt + 1)
                    elif i + 1 < len(seq):
                        emit_qk(i + 1, 0)
                    nc.tensor.matmul(
                        pav[0:33, :], vts_all[im][kt][:, 33 * hA:33 * hA + 33],
                        est[:, 0:512], start=(kt == 0), stop=(kt == 7),
                        tile_position=(0, 0),
                    )
                    nc.tensor.matmul(
                        pav[64:97, :], vts_all[im][kt][:, 33 * hB:33 * hB + 33],
                        est[:, 512:1024], start=(kt == 0), stop=(kt == 7),
                        tile_position=(0, 64),
                    )
                    if kt == 2 and norm_pending:
                        emit_norm(*norm_pending.pop(0))
                norm_pending.append((i, pav))
            emit_norm(*norm_pending.pop(0))

            # ================= phase 3: pos-enc add + projection ==========
            for im in range(B_LOC):
                for ct in range(2):
                    nc.vector.tensor_tensor(attn_all[im][ct][:],
                                            attn_all[im][ct][:],
                                            peacc_all[im][ct][:], op=ALU.add)
                for ot in range(2):
                    ot_sb = outpool.tile([128, N], F32, tag="osb",
                                         name="ot_sb")
                    for ncol in range(2):
                        pp = ps_big.tile([128, 512], F32, tag="big", name="pp")
                        for ct in range(2):
                            nc.tensor.matmul(
                                pp[:],
                                w2t[ct][:, 128 * ot:128 * (ot + 1)],
                                attn_all[im][ct][:, 512 * ncol:512 * (ncol + 1)],
                                start=(ct == 0), stop=(ct == 1),
                            )
                        nc.vector.tensor_scalar_add(
                            ot_sb[:, 512 * ncol:512 * (ncol + 1)], pp[:],
                            b2[ot][:])
                    nc.sync.dma_start(
                        out_d[im, 128 * ot:128 * (ot + 1), :], ot_sb[:])
    _split_waits(nc)
    return nc


def kernel(**inputs):
    x = np.asarray(inputs['x'], np.float32)
    Bful, C, Hh, Ww = x.shape
    assert (Bful, C, Hh, Ww) == (B, DIM, H, W)

    key = 'prog'
    if key not in _cache:
        _cache[key] = _build_program()
    nc = _cache[key]

    wd = _prep_weights(
        np.asarray(inputs['qkv_w'], np.float32), np.asarray(inputs['qkv_gamma'], np.float32),
        np.asarray(inputs['qkv_beta'], np.float32), np.asarray(inputs['qkv_mean'], np.float32),
        np.asarray(inputs['qkv_var'], np.float32),
        np.asarray(inputs['pe_w'], np.float32), np.asarray(inputs['pe_gamma'], np.float32),
        np.asarray(inputs['pe_beta'], np.float32), np.asarray(inputs['pe_mean'], np.float32),
        np.asarray(inputs['pe_var'], np.float32),
        np.asarray(inputs['proj_w'], np.float32), np.asarray(inputs['proj_gamma'], np.float32),
        np.asarray(inputs['proj_beta'], np.float32), np.asarray(inputs['proj_mean'], np.float32),
        np.asarray(inputs['proj_var'], np.float32),
    )

    xr = x.reshape(B, 256, N).astype(ml_dtypes.bfloat16)
    in_maps = []
    for c in range(N_CORES):
        m = dict(wd)
        m['x'] = xr[B_LOC * c:B_LOC * (c + 1)]
        in_maps.append(m)

    res = run_bass_kernel_spmd(nc, in_maps, list(range(N_CORES)))
    out = np.concatenate([r['out'] for r in res.results], axis=0)
    return out.reshape(B, 256, H, W).astype(np.float32)


def make_runner(**inputs):
    """Build (jitted_fn, concat_inputs, zero_outs, postprocess) for benchmarking.

    Mirrors bass2jax.run_bass_via_pjrt's multi-core path but without donation
    so the same buffers can be re-executed for timing."""
    import jax
    from jax.sharding import Mesh, PartitionSpec
    from jax.experimental.shard_map import shard_map
    from concourse import bass2jax, mybir as _mb

    x = np.asarray(inputs['x'], np.float32)
    wd = _prep_weights(**{k: np.asarray(inputs[k], np.float32) for k in (
        'qkv_w', 'qkv_gamma', 'qkv_beta', 'qkv_mean', 'qkv_var',
        'pe_w', 'pe_gamma', 'pe_beta', 'pe_mean', 'pe_var',
        'proj_w', 'proj_gamma', 'proj_beta', 'proj_mean', 'proj_var')})
    if 'prog' not in _cache:
        _cache['prog'] = _build_program()
    nc = _cache['prog']
    xr = x.reshape(B, 256, N).astype(ml_dtypes.bfloat16)
    in_maps = []
    for c in range(N_CORES):
        m = dict(wd)
        m['x'] = xr[B_LOC * c:B_LOC * (c + 1)]
        in_maps.append(m)

    bass2jax.install_neuronx_cc_hook()
    in_names, out_names, out_avals, zero_outs = [], [], [], []
    for alloc in nc.m.functions[0].allocations:
        if not isinstance(alloc, _mb.MemoryLocationSet):
            continue
        name = alloc.memorylocations[0].name
        if alloc.kind == "ExternalInput":
            if nc.partition_id_tensor and name == nc.partition_id_tensor.name:
                continue
            in_names.append(name)
        elif alloc.kind == "ExternalOutput":
            out_names.append(name)
            out_avals.append(jax.core.ShapedArray(
                tuple(alloc.tensor_shape), _mb.dt.np(alloc.dtype)))
            zero_outs.append(np.zeros(tuple(alloc.tensor_shape),
                                      _mb.dt.np(alloc.dtype)))
    n_params = len(in_names)
    all_names = in_names + out_names

    pname = nc.partition_id_tensor.name if nc.partition_id_tensor else None

    def _body(*args):
        operands = list(args)
        names = list(all_names)
        if pname is not None:
            operands.append(bass2jax.partition_id_tensor())
            names.append(pname)
        outs = bass2jax._bass_exec_p.bind(
            *operands,
            out_avals=tuple(out_avals),
            in_names=tuple(names),
            out_names=tuple(out_names),
            lowering_input_output_aliases=(),
            sim_require_finite=True,
            sim_require_nnan=True,
            nc=nc,
        )
        return tuple(outs)

    devices = jax.devices()[:N_CORES]
    mesh = Mesh(np.asarray(devices), ("core",))
    nin = n_params + len(out_names)
    sharded = jax.jit(
        shard_map(_body, mesh=mesh,
                  in_specs=(PartitionSpec("core"),) * nin,
                  out_specs=(PartitionSpec("core"),) * len(out_names),
                  check_rep=False),
        keep_unused=True,
    )
    per_core = [[np.asarray(m[nm]) for nm in in_names] for m in in_maps]
    concat_in = [np.concatenate([per_core[c][i] for c in range(N_CORES)], axis=0)
                 for i in range(n_params)]
    concat_zeros = [np.zeros((N_CORES * z.shape[0], *z.shape[1:]), z.dtype)
                    for z in zero_outs]

    def post(out_arrs):
        full = np.asarray(out_arrs[0]).reshape(N_CORES, B_LOC, 256, N)
        return full.reshape(B, 256, N).reshape(B, 256, H, W).astype(np.float32)

    return sharded, concat_in, concat_zeros, post
